# revision 1
# baseline (speedup 1.0000x reference)
"""BCJR detector kernel for Trainium2, 8-core batch-parallel.

Layout per core: 128 words on SBUF partitions, 16 trellis states on the
free dim.  Trellis structure (derived from reference._trellis):
  alpha:  a'[st] = (a[st>>1] + a[(st>>1)+8]) * g[st]
  beta:   b'[s]  = (b[2s%16] + b[2s%16+1])  * g[s]
Both gathers are step-0 broadcast access patterns, no real gather needed.
alpha/beta are kept unnormalized with a lazy per-partition rescale every
NORM steps (decisions are invariant to per-(word,t) positive scaling; an
all-underflow row propagates zeros/NaN and decodes to 0 exactly like the
reference's NaN cascade).
"""

import math
import sys

import numpy as np

sys.path.insert(0, "/opt/trn_rl_repo")

B, T, S, MEM, V = 1024, 2048, 16, 4, 4
NCORES = 8
BPC = B // NCORES  # 128 words per core
BLK = 128          # t-steps per g/combine block
NORM = 16          # rescale cadence


def _build(nc, Tn, g_scale, g_bias):
    import concourse.bass as bass  # noqa: F401
    from concourse import mybir, tile
    from concourse.alu_op_type import AluOpType as OP
    from concourse.mybir import ActivationFunctionType as AF

    dt = mybir.dt.float32
    nblk = Tn // BLK

    # packed input: cols [0:Tn]=y, [Tn:Tn+S]=sp
    yin_d = nc.dram_tensor("yin", [BPC, Tn + S], dt, kind="ExternalInput")
    out_d = nc.dram_tensor("dec", [BPC, Tn], dt, kind="ExternalOutput")

    with tile.TileContext(nc) as tc:
        with (
            tc.tile_pool(name="big", bufs=1) as big,
            tc.tile_pool(name="gp", bufs=2) as gp,
            tc.tile_pool(name="sm", bufs=1) as sm,
        ):
            yin_sb = big.tile([BPC, Tn + S], dt, tag="y")
            y_sb = yin_sb[:, 0:Tn]
            sp_sb = yin_sb[:, Tn : Tn + S]
            H = S // 2
            ACH = 1024  # c-store chunk (t-steps) to keep AP offsets small
            cstores = [
                big.tile(
                    [BPC, H * min(ACH, Tn)], dt,
                    name=f"cstore{i}", tag=f"cstore{i}",
                )
                for i in range((Tn + ACH - 1) // ACH)
            ]

            def csl_of(t):
                c = cstores[t // ACH]
                k = t % ACH
                return c[:, k * H : (k + 1) * H]
            bstore = big.tile([BPC, S * BLK], dt, tag="bstore")
            wtile = big.tile([BPC, S * BLK], dt, tag="w")
            dtile = big.tile([BPC, (S // 2) * BLK], dt, tag="dtile")
            upt = sm.tile([BPC, BLK], dt, tag="up")
            dec = sm.tile([BPC, BLK], dt, tag="dec")
            carry = sm.tile([BPC, S], dt, tag="carry")
            c_a = sm.tile([BPC, S], dt, tag="c_a")
            c_b = sm.tile([BPC, S], dt, tag="c_b")
            r_a = sm.tile([BPC, 1], dt, tag="r_a")
            r_b = sm.tile([BPC, 1], dt, tag="r_b")
            s_a = sm.tile([BPC, 1], dt, tag="s_a")
            s_b = sm.tile([BPC, 1], dt, tag="s_b")
            bias_t = sm.tile([BPC, 1], dt, tag="bias")
            nc.vector.memset(bias_t[:, :], float(g_bias))

            nc.sync.dma_start(yin_sb[:, :], yin_d[:, :])

            def gen_g(blk, which):
                """g[:, k*16+s] = exp(scale*(y[t0+k]-sp[s])^2 + bias) for k in blk."""
                g = gp.tile([BPC, S * BLK], dt, tag=f"g{which}")
                t0 = blk * BLK
                yv = (
                    y_sb[:, t0 : t0 + BLK]
                    .unsqueeze(2)
                    .broadcast_to((BPC, BLK, S))
                )
                spv = sp_sb[:, :].unsqueeze(1).broadcast_to((BPC, BLK, S))
                d3 = g[:, :].rearrange("p (k s) -> p k s", s=S)
                nc.gpsimd.tensor_tensor(d3, yv, spv, OP.subtract)
                nc.gpsimd.tensor_tensor(d3, d3, d3, OP.mult)
                nc.scalar.activation(
                    g[:, :], g[:, :], AF.Exp,
                    bias=bias_t[:, :], scale=float(g_scale),
                )
                return g

            # ---------------- alpha pass (forward), pairsum (c) form ------
            # c_t[j] = alpha_t[j] + alpha_t[j+8]  (8 wide); alpha_{t+1} =
            # c_t[s>>1] * g_t[s] materialized transiently in c_a.
            nc.vector.memset(csl_of(0), 0.0)
            nc.vector.memset(cstores[0][:, 0:1], 1.0)
            nc.vector.memset(r_a[:, :], 1.0)
            nc.vector.memset(r_b[:, :], 1.0)
            for blk in range(nblk):
                g = gen_g(blk, "a")
                for k in range(BLK):
                    t = blk * BLK + k
                    if t >= Tn - 1:
                        break
                    cv = (
                        csl_of(t)
                        .unsqueeze(2)
                        .broadcast_to((BPC, 8, 2))
                    )
                    g3 = g[:, k * S : (k + 1) * S].rearrange(
                        "p (a b) -> p a b", b=2
                    )
                    a3 = c_a[:, :].rearrange("p (a b) -> p a b", b=2)
                    if t % NORM == NORM - 1:
                        nc.vector.scalar_tensor_tensor(
                            a3, cv, r_a[:, :], g3, OP.mult, OP.mult,
                            accum_out=s_a[:, :],
                        )
                        nc.vector.reciprocal(r_a[:, :], s_a[:, :])
                    else:
                        nc.vector.tensor_tensor(a3, cv, g3, OP.mult)
                    nc.vector.tensor_tensor(
                        csl_of(t + 1), c_a[:, 0:8], c_a[:, 8:16], OP.add
                    )

            # ---------------- beta pass (backward) + combine ----------------
            for blk in range(nblk - 1, -1, -1):
                g = gen_g(blk, "b")
                for k in range(BLK - 1, -1, -1):
                    t = blk * BLK + k
                    if t == Tn - 1:
                        bprev = None  # init state
                    elif k == BLK - 1:
                        bprev = carry[:, :]
                    else:
                        bprev = bstore[:, (k + 1) * S : (k + 2) * S]
                    bout = bstore[:, k * S : (k + 1) * S]
                    o3 = bout.rearrange("p (a b) -> p a b", a=2)
                    g3 = g[:, k * S : (k + 1) * S].rearrange(
                        "p (a b) -> p a b", a=2
                    )
                    if bprev is None:
                        # b = init [1,0,...,0]; b' [s] = (init[2s%16]+init[2s%16+1])*g
                        # = g[s] if s in {0,8} else 0
                        nc.vector.memset(bout, 0.0)
                        nc.vector.tensor_tensor(
                            bout[:, 0:9:8],
                            g[:, k * S : k * S + 9 : 8],
                            g[:, k * S : k * S + 9 : 8],
                            OP.max,
                        )
                        continue
                    vE = bprev[:, 0:16:2].unsqueeze(1).broadcast_to((BPC, 2, 8))
                    vO = bprev[:, 1:16:2].unsqueeze(1).broadcast_to((BPC, 2, 8))
                    c3 = c_b[:, :].rearrange("p (a b) -> p a b", a=2)
                    nc.vector.tensor_tensor(c3, vE, vO, OP.add)
                    if t % NORM == NORM - 1:
                        nc.vector.scalar_tensor_tensor(
                            o3, c3, r_b[:, :], g3, OP.mult, OP.mult,
                            accum_out=s_b[:, :],
                        )
                        nc.vector.reciprocal(r_b[:, :], s_b[:, :])
                    else:
                        nc.vector.tensor_tensor(o3, c3, g3, OP.mult)
                # save carry for next (lower) block before combine overwrites
                nc.vector.tensor_copy(carry[:, :], bstore[:, 0:S])
                # combine in pairsum form:
                #   up-dn = sum_j c[j] * (w[2j] - w[2j+1]),  w = g*beta
                nc.gpsimd.tensor_tensor(wtile[:, :], g[:, :], bstore[:, :], OP.mult)
                t0 = blk * BLK
                w3 = wtile[:, :].rearrange("p (k s) -> p k s", s=S)
                d3 = dtile[:, :].rearrange("p (k j) -> p k j", j=8)
                nc.gpsimd.tensor_tensor(
                    d3, w3[:, :, 0:16:2], w3[:, :, 1:16:2], OP.subtract
                )
                cch = cstores[t0 // ACH]
                k0 = t0 % ACH
                c3 = cch[:, k0 * H : (k0 + BLK) * H].rearrange(
                    "p (k j) -> p k j", j=8
                )
                nc.gpsimd.tensor_tensor(d3, d3, c3, OP.mult)
                nc.vector.tensor_reduce(
                    upt[:, :], d3, mybir.AxisListType.X, OP.add,
                )
                nc.vector.tensor_scalar(
                    dec[:, :], upt[:, :], 0.0, None, OP.is_lt,
                )
                nc.sync.dma_start(out_d[:, t0 : t0 + BLK], dec[:, :])
    return nc


def _legalize_multiwait(bir):
    """Engine instruction structs embed at most ONE sem wait.  Tile's engine
    queue-depth throttle adds a self-wait to nearly every DVE instruction, so
    instructions that also need a cross-engine wait end up with two and
    walrus rejects them.  Split: move all waits onto a 1-element Memset
    carrier inserted just before (same engine, in-order), leaving the real
    instruction wait-free."""
    n = 0
    for fn in bir["functions"]:
        for blk in fn["blocks"]:
            newl = []
            for inst in blk["instructions"]:
                si = inst.get("sync_info") or {}
                waits = si.get("on_wait") or []
                eng = inst.get("engine")
                if len(waits) >= 2 and eng in (
                    "DVE", "Pool", "Activation", "PE", "SP",
                ):
                    for j, w in enumerate(waits):
                        carrier = {
                            "name": inst["name"] + f"-wc{j}",
                            "opcode": "EventSemaphore",
                            "engine": eng,
                            "ins": [],
                            "outs": [],
                            "sync_info": {"on_wait": [w], "on_update": []},
                        }
                        if "debug" in inst:
                            carrier["debug"] = inst["debug"]
                        newl.append(carrier)
                        n += 1
                    si["on_wait"] = []
                    inst["sync_info"] = si
                newl.append(inst)
            blk["instructions"] = newl
    return n


def _finalize(nc):
    """Apply the multi-wait legalization and pin the serialized BIR."""
    import json as _json

    bir = _json.loads(nc.to_json_bytes())
    _legalize_multiwait(bir)
    bts = _json.dumps(bir).encode()
    nc.to_json_bytes = lambda: bts
    return nc


def _np_f32(x):
    return np.ascontiguousarray(np.asarray(x, dtype=np.float32))


def kernel(y, h, snr):
    import concourse.bass as bass
    from concourse.bass_utils import run_bass_kernel_spmd

    y = _np_f32(y)
    h = _np_f32(h)
    snr_f = float(np.asarray(snr))
    sigma = np.float32(10.0 ** (-snr_f / 10.0))

    bits = (np.arange(S)[:, None] >> np.arange(MEM - 1, -1, -1)) & 1
    syms = (1 - 2 * bits).astype(np.float32)          # [S, MEM]
    sp = (syms @ h[:, ::-1].T).astype(np.float32)     # [S, V]
    sp_b = sp.T[np.arange(BPC) % V].astype(np.float32)  # [BPC, S], same per core

    scale = np.float32(-1.0 / (2.0 * sigma * sigma))
    bias = np.float32(-math.log(math.sqrt(2.0 * math.pi) * sigma))

    nc = bass.Bass()
    _build(nc, T, scale, bias)
    _finalize(nc)

    in_maps = [
        {
            "yin": np.ascontiguousarray(
                np.concatenate([y[c * BPC : (c + 1) * BPC], sp_b], axis=1)
            ),
        }
        for c in range(NCORES)
    ]
    res = run_bass_kernel_spmd(nc, in_maps, core_ids=list(range(NCORES)))
    dec = np.concatenate([r["dec"] for r in res.results], axis=0)  # [B, T]

    out = np.zeros((B, T), np.float32)
    out[:, MEM - 1 :] = dec[:, : T - (MEM - 1)]
    return out



# revision 29
# speedup vs baseline: 5.4555x; 5.4555x over previous
"""BCJR detector kernel for Trainium2, 8-core batch-parallel, chunk-parallel time axis.

Layout per core: 128 words on SBUF partitions.  Each word's T=2048 trellis
steps are split into C=32 independent chunks of K=64 steps processed
simultaneously in the free dimension (W warm-up steps absorb the window
boundary), so each recursion step is one wide vector op over all chunks
instead of 16 elements.  Trellis structure (reference._trellis):
  alpha:  A_t[s] = cs_t[s>>1] * g_t[s];  cs_{t+1}[j] = A_t[j] + A_t[j+8]
  beta:   B_t[s] = e_{t+1}[s%8] * g_t[s]; e_t[j]    = B_t[2j] + B_t[2j+1]
  up_t   = sum_{s even} A_t[s]*B_t[s],  dn_t = sum_{s odd}, bit = up < dn
g is the per-t-uniformly-rescaled likelihood ghat[s] = exp(y*u_s + v_s)
(uniform positive per-t scaling leaves every up<dn decision unchanged).
alpha/beta run unnormalized with a lazy per-chunk rescale every NORM steps;
a (word,chunk) whose values under/overflow propagates zeros/NaN and its
decisions decode to 0.  Per-word alive flags off the final chunk states
then zero any word with a dead chunk, reproducing the reference's global
0/0 -> NaN cascade semantics on peaked-likelihood inputs.
Engine split: ghat on ACT; alpha + 8 beta chunks on DVE; 24 beta chunks on
Pool (2-free-dim scalar_tensor_tensor halves keep walrus happy); combine
windows split DVE/Pool; strided DMA-out per window on the SP queue.
"""

import math
import sys

import numpy as np

sys.path.insert(0, "/opt/trn_rl_repo")

B, T, S, MEM, V = 1024, 2048, 16, 4, 4
NCORES = 8
BPC = B // NCORES   # 128 words per core
C = 32              # time chunks per word
K = T // C          # 64 steps per chunk
W = 8               # warm-up steps
NSTEPS = K + W      # 72
NORM = 32           # lazy rescale cadence
CP = 32             # beta chunks on Pool (all of them)
KBD = 2             # combine window steps, DVE windows (slots [SLOTP, K))
KBP = 4             # combine window steps, Pool windows (slots [0, SLOTP))
SLOTP = 40          # slot boundary: Pool combines [0, SLOTP), DVE the rest


def _build(nc, Tn):
    import concourse.bass as bass  # noqa: F401
    from concourse import mybir, tile
    from concourse.alu_op_type import AluOpType as OP
    from concourse.mybir import ActivationFunctionType as AF

    f32 = mybir.dt.float32
    bf16 = mybir.dt.bfloat16
    GT = Tn + 2 * W
    CS = C * S  # 512 elems per step-slot

    y_d = nc.dram_tensor("y", [BPC, Tn], f32, kind="ExternalInput")
    uv_d = nc.dram_tensor("uv", [BPC, 2 * S], f32, kind="ExternalInput")
    # dec DRAM layout is slot-major: linear index = slot*C + c; the host
    # un-permutes to t = c*K + slot with a free numpy transpose.
    out_d = nc.dram_tensor("dec", [BPC, Tn], f32, kind="ExternalOutput")

    with tile.TileContext(nc) as tc:
        with (
            tc.tile_pool(name="main", bufs=1) as mp,
            tc.tile_pool(name="ps", bufs=1, space="PSUM") as pp,
        ):
            y_sb = mp.tile([BPC, Tn], f32, tag="y")
            uv_sb = mp.tile([BPC, 2 * S], f32, tag="uv")
            g_sb = mp.tile([BPC, GT * S], bf16, tag="g")
            A_sb = mp.tile([BPC, K * CS], bf16, tag="A")
            B_sb = mp.tile([BPC, K * CS], bf16, tag="B")
            cs = mp.tile([BPC, C * 8], bf16, tag="cs")
            ev = mp.tile([BPC, C * 8], bf16, tag="ev")
            s_a = mp.tile([BPC, C], f32, tag="s_a")
            r_a = mp.tile([BPC, C], f32, tag="r_a")
            t1b = mp.tile([BPC, CP * 4], bf16, tag="t1b")
            t2b = mp.tile([BPC, CP * 2], bf16, tag="t2b")
            sbb = mp.tile([BPC, CP], bf16, tag="sbb")
            r_b = mp.tile([BPC, CP], f32, tag="r_b")
            aA = mp.tile([BPC, C], f32, tag="aA")
            aB = mp.tile([BPC, C], f32, tag="aB")
            kp = mp.tile([BPC, 2], f32, tag="kp")
            # DVE combine scratch, carved from y like the Pool scratch
            mwd = y_sb[:, 1536:2048].bitcast(bf16)
            dec = mp.tile([BPC, Tn], bf16, tag="dec")
            # Pool combine scratch carved out of y_sb (y is dead after
            # ghat-gen; Tile's WAR deps order the reuse)
            mwp = y_sb[:, 0:1024].bitcast(bf16)   # KBP*CS bf16
            t1p = y_sb[:, 1024:1536].bitcast(bf16)  # tree scratch

            nc.sync.dma_start(y_sb[:, :], y_d[:, :])
            nc.sync.dma_start(uv_sb[:, :], uv_d[:, :])
            u_sb = uv_sb[:, 0:S]
            v_sb = uv_sb[:, S : 2 * S]

            gv = g_sb[:, :].rearrange("p (t s) -> p t s", s=S)
            # warm-up delta pattern g=[1,0,...]: keeps chunk 0's (alpha) and
            # the last chunk's (beta) exact delta init frozen across warm steps
            nc.vector.memset(g_sb[:, 0 : W * S], 0.0)
            nc.vector.memset(gv[:, 0:W, 0:1], 1.0)
            nc.vector.memset(g_sb[:, (W + Tn) * S :], 0.0)
            nc.vector.memset(gv[:, W + Tn :, 0:1], 1.0)

            # ---- ghat[t,s] = exp(y_t * u_s + v_s), bf16, t-major ----------
            # generated in two k-bands over the (chunk, step) comb so alpha
            # can start after band A and beta after band B
            gv4 = g_sb[:, W * S : (W + Tn) * S].rearrange(
                "p (c k s) -> p c k s", c=C, k=K, s=S
            )
            y4 = y_sb[:, :].rearrange("p (c k) -> p c k", c=C)
            KH = K // 2
            for k0, k1 in ((0, K),):
                for s in range(S):
                    nc.scalar.activation(
                        gv4[:, :, k0:k1, s : s + 1],
                        y4[:, :, k0:k1].unsqueeze(3), AF.Exp,
                        bias=v_sb[:, s : s + 1], scale=u_sb[:, s : s + 1],
                    )

            # ---- recursion state init ------------------------------------
            cs3 = cs[:, :].rearrange("p (c j) -> p c j", j=8)
            nc.vector.memset(cs[:, :], 1.0)
            nc.vector.memset(cs[:, 0:8], 0.0)
            nc.vector.memset(cs[:, 0:1], 1.0)
            ev3 = ev[:, :].rearrange("p (c j) -> p c j", j=8)
            nc.vector.memset(ev[:, :], 1.0)
            nc.vector.memset(ev[:, (C - 1) * 8 :], 0.0)
            nc.vector.memset(ev[:, (C - 1) * 8 : (C - 1) * 8 + 1], 1.0)

            cbc = cs3.unsqueeze(3).broadcast_to((BPC, C, 8, 2))
            evp = ev3[:, 0:CP, :]
            ebc = ev3.unsqueeze(2).broadcast_to((BPC, C, 2, 8))
            t1v = t1b[:, :].rearrange("p (c x) -> p c x", x=4)
            t2v = t2b[:, :].rearrange("p (c x) -> p c x", x=2)

            # ---- passes: alpha fwd (DVE) + beta bwd (Pool, stt halves),
            # interleaved per iteration -------------------------------------
            for i in range(NSTEPS):
                ka = i
                kb = NSTEPS - 1 - i
                # alpha step ka (DVE)
                if ka < W:
                    aslot = A_sb[:, 0:CS]  # scratch; rewritten at ka == W
                else:
                    aslot = A_sb[:, (ka - W) * CS : (ka - W + 1) * CS]
                o3 = aslot.rearrange("p (c j r) -> p c j r", j=8, r=2)
                g3 = gv[:, ka : ka + (C - 1) * K + 1 : K, :].rearrange(
                    "p c (j r) -> p c j r", r=2
                )
                nc.vector.tensor_tensor(o3, cbc, g3, OP.mult)
                a3 = aslot.rearrange("p (c s) -> p c s", s=S)
                nc.vector.tensor_tensor(
                    cs3, a3[:, :, 0:8], a3[:, :, 8:16], OP.add
                )
                # beta step kb (Pool), all chunks: two 2-free-dim halves
                if kb >= K:
                    bslot = B_sb[:, (K - 1) * CS :]  # scratch until kb==K-1
                else:
                    bslot = B_sb[:, kb * CS : (kb + 1) * CS]
                b3p = bslot.rearrange("p (c s) -> p c s", s=S)
                o3p = bslot.rearrange("p (c h j) -> p c h j", h=2, j=8)
                g3p = gv[:, W + kb : W + kb + (C - 1) * K + 1 : K, :].rearrange(
                    "p c (h j) -> p c h j", j=8
                )
                nc.gpsimd.tensor_tensor(o3p, ebc, g3p, OP.mult)
                nc.gpsimd.tensor_tensor(
                    evp, b3p[:, :, 0:16:2], b3p[:, :, 1:16:2], OP.add
                )
                if i % NORM == NORM - 1:
                    # alpha rescale (DVE)
                    nc.vector.tensor_reduce(
                        s_a[:, :], cs3, mybir.AxisListType.X, OP.add
                    )
                    nc.vector.reciprocal(r_a[:, :], s_a[:, :])
                    nc.vector.tensor_tensor(
                        cs3, cs3,
                        r_a[:, :].unsqueeze(2).broadcast_to((BPC, C, 8)),
                        OP.mult,
                    )
                    # beta rescale: Pool TT tree, DVE reciprocal, Pool apply
                    nc.gpsimd.tensor_tensor(
                        t1v, evp[:, :, 0:4], evp[:, :, 4:8], OP.add
                    )
                    nc.gpsimd.tensor_tensor(
                        t2v, t1v[:, :, 0:2], t1v[:, :, 2:4], OP.add
                    )
                    nc.gpsimd.tensor_tensor(
                        sbb[:, :].rearrange("p (c x) -> p c x", x=1),
                        t2v[:, :, 0:1], t2v[:, :, 1:2], OP.add,
                    )
                    nc.vector.reciprocal(r_b[:, :], sbb[:, :])
                    nc.gpsimd.tensor_tensor(
                        evp, evp,
                        r_b[:, :].unsqueeze(2).broadcast_to((BPC, CP, 8)),
                        OP.mult,
                    )

            # ---- alive flags + global kill factor (DVE) -------------------
            nc.vector.tensor_reduce(s_a[:, :], cs3, mybir.AxisListType.X, OP.add)
            nc.vector.tensor_scalar(aA[:, :], s_a[:, :], 0.0, None, OP.is_gt)
            nc.vector.tensor_reduce(aB[:, :], ev3, mybir.AxisListType.X, OP.add)
            nc.vector.tensor_scalar(aB[:, :], aB[:, :], 0.0, None, OP.is_gt)
            nc.vector.tensor_reduce(
                kp[:, 0:1], aA[:, :], mybir.AxisListType.X, OP.min
            )
            nc.vector.tensor_reduce(
                kp[:, 1:2], aB[:, :], mybir.AxisListType.X, OP.min
            )
            nc.vector.tensor_tensor(kp[:, 0:1], kp[:, 0:1], kp[:, 1:2], OP.mult)

            # ---- combine: bit_t = (sum_even A*B < sum_odd A*B) -----------
            # reduction: d[j] = m[2j] - m[2j+1]; halves-tree sum of d (8->1);
            # bit = (sum < 0).  dec is slot-major: index = slot*C + c.
            m3d = mwd.rearrange("p (n s) -> p n s", s=S)
            d8d = mwd[:, 0 : KBD * C * 8].rearrange("p (n j) -> p n j", j=8)
            for w in range((K - SLOTP) // KBD):
                k0 = K - (w + 1) * KBD
                awin = A_sb[:, k0 * CS : (k0 + KBD) * CS]
                bwin = B_sb[:, k0 * CS : (k0 + KBD) * CS]
                nc.vector.tensor_tensor(mwd, awin, bwin, OP.mult)
                # pair-diff into the low half of mwd (reads lane 2j, 2j+1)
                nc.vector.tensor_tensor(
                    d8d, m3d[:, :, 0:16:2], m3d[:, :, 1:16:2], OP.subtract
                )
                nw = KBD * C
                d4 = mwd[:, 0 : nw * 4].rearrange("p (n j) -> p n j", j=4)
                d8v = mwd[:, 0 : nw * 8].rearrange("p (n j) -> p n j", j=8)
                nc.vector.tensor_tensor(d4, d8v[:, :, 0:4], d8v[:, :, 4:8], OP.add)
                d2 = mwd[:, 0 : nw * 2].rearrange("p (n j) -> p n j", j=2)
                d4v = mwd[:, 0 : nw * 4].rearrange("p (n j) -> p n j", j=4)
                nc.vector.tensor_tensor(d2, d4v[:, :, 0:2], d4v[:, :, 2:4], OP.add)
                d1 = mwd[:, 0:nw]
                d2v = mwd[:, 0 : nw * 2].rearrange("p (n j) -> p n j", j=2)
                nc.vector.tensor_tensor(
                    d1.rearrange("p (n j) -> p n j", j=1),
                    d2v[:, :, 0:1], d2v[:, :, 1:2], OP.add,
                )
                nc.vector.tensor_scalar(
                    dec[:, k0 * C : (k0 + KBD) * C], d1, 0.0, None, OP.is_lt
                )
            m3p = mwp.rearrange("p (n s) -> p n s", s=S)
            npw = KBP * C
            d8p = t1p[:, 0 : npw * 8].rearrange("p (n j) -> p n j", j=8)
            for w in range(SLOTP // KBP):
                k0 = SLOTP - (w + 1) * KBP
                awin = A_sb[:, k0 * CS : (k0 + KBP) * CS]
                bwin = B_sb[:, k0 * CS : (k0 + KBP) * CS]
                nc.gpsimd.tensor_tensor(mwp, awin, bwin, OP.mult)
                nc.gpsimd.tensor_tensor(
                    d8p, m3p[:, :, 1:16:2], m3p[:, :, 0:16:2], OP.subtract
                )
                # d8p holds -d; bit = (sum > 0) on the negated sum
                d4 = t1p[:, 0 : npw * 4].rearrange("p (n j) -> p n j", j=4)
                d8v = t1p[:, 0 : npw * 8].rearrange("p (n j) -> p n j", j=8)
                nc.gpsimd.tensor_tensor(d4, d8v[:, :, 0:4], d8v[:, :, 4:8], OP.add)
                d2 = t1p[:, 0 : npw * 2].rearrange("p (n j) -> p n j", j=2)
                d4v = t1p[:, 0 : npw * 4].rearrange("p (n j) -> p n j", j=4)
                nc.gpsimd.tensor_tensor(d2, d4v[:, :, 0:2], d4v[:, :, 2:4], OP.add)
                d1 = t1p[:, 0:npw]
                d2v = t1p[:, 0 : npw * 2].rearrange("p (n j) -> p n j", j=2)
                nc.gpsimd.tensor_tensor(
                    d1.rearrange("p (n j) -> p n j", j=1),
                    d2v[:, :, 0:1], d2v[:, :, 1:2], OP.add,
                )
                nc.gpsimd.tensor_scalar(
                    dec[:, k0 * C : (k0 + KBP) * C], d1, 0.0, None, OP.is_gt
                )

            # ---- global kill + output -------------------------------------
            nc.vector.tensor_scalar(dec[:, :], dec[:, :], kp[:, 0:1], None, OP.mult)
            nc.gpsimd.dma_start(out_d[:, :], dec[:, :])
    return nc


def _legalize_multiwait(bir):
    """Engine instruction structs embed at most ONE sem wait.  Tile's engine
    queue-depth throttle adds a self-wait to nearly every DVE instruction, so
    instructions that also need a cross-engine wait end up with two and
    walrus rejects them.  Split: move all waits onto a 1-element Memset
    carrier inserted just before (same engine, in-order), leaving the real
    instruction wait-free."""
    n = 0
    for fn in bir["functions"]:
        for blk in fn["blocks"]:
            newl = []
            for inst in blk["instructions"]:
                si = inst.get("sync_info") or {}
                waits = si.get("on_wait") or []
                eng = inst.get("engine")
                if len(waits) >= 2 and eng in (
                    "DVE", "Pool", "Activation", "PE", "SP",
                ):
                    for j, w in enumerate(waits):
                        carrier = {
                            "name": inst["name"] + f"-wc{j}",
                            "opcode": "EventSemaphore",
                            "engine": eng,
                            "ins": [],
                            "outs": [],
                            "sync_info": {"on_wait": [w], "on_update": []},
                        }
                        if "debug" in inst:
                            carrier["debug"] = inst["debug"]
                        newl.append(carrier)
                        n += 1
                    si["on_wait"] = []
                    inst["sync_info"] = si
                newl.append(inst)
            blk["instructions"] = newl
    return n


def _finalize(nc):
    import json as _json

    bir = _json.loads(nc.to_json_bytes())
    _legalize_multiwait(bir)
    bts = _json.dumps(bir).encode()
    nc.to_json_bytes = lambda: bts
    return nc


def _np_f32(x):
    return np.ascontiguousarray(np.asarray(x, dtype=np.float32))


def kernel(y, h, snr):
    import concourse.bass as bass
    from concourse.bass_utils import run_bass_kernel_spmd

    y = _np_f32(y)
    h = _np_f32(h)
    snr_f = float(np.asarray(snr))
    sigma = np.float32(10.0 ** (-snr_f / 10.0))

    bits = (np.arange(S)[:, None] >> np.arange(MEM - 1, -1, -1)) & 1
    syms = (1 - 2 * bits).astype(np.float32)          # [S, MEM]
    sp = (syms @ h[:, ::-1].T).astype(np.float32)     # [S, V]
    sp_b = sp.T[np.arange(BPC) % V].astype(np.float32)  # [BPC, S]

    scale = np.float32(-1.0 / (2.0 * sigma * sigma))
    bias = np.float32(-math.log(math.sqrt(2.0 * math.pi) * sigma))
    # ghat = exp(y*u + v): likelihood with the per-t uniform factor dropped
    u_b = (np.float32(-2.0) * scale * sp_b).astype(np.float32)
    v_b = (scale * sp_b * sp_b + bias).astype(np.float32)
    uv = np.ascontiguousarray(np.concatenate([u_b, v_b], axis=1))

    nc = bass.Bass()
    _build(nc, T)
    _finalize(nc)

    in_maps = [
        {
            "y": np.ascontiguousarray(y[c * BPC : (c + 1) * BPC]),
            "uv": uv,
        }
        for c in range(NCORES)
    ]
    res = run_bass_kernel_spmd(nc, in_maps, core_ids=list(range(NCORES)))
    dec = np.concatenate([r["dec"] for r in res.results], axis=0)  # [B, T]
    dec = np.asarray(dec, dtype=np.float32)
    dec = np.nan_to_num(dec, nan=0.0, posinf=0.0, neginf=0.0)
    # slot-major -> t-major: t = c*K + slot
    dec = dec.reshape(B, K, C).transpose(0, 2, 1).reshape(B, T)

    out = np.zeros((B, T), np.float32)
    out[:, MEM - 1 :] = dec[:, : T - (MEM - 1)]
    return out
